# revision 4
# baseline (speedup 1.0000x reference)
"""Binary-weight 3x3 SAME conv (NHWC) on Trainium2, data-parallel over 8 cores.

Problem: x (32,56,56,256) f32, w (3,3,256,256) f32.
  out = conv2d(x, sign(clip(w,-1,1)), SAME, stride 1)   # NHWC / HWIO

v2 strategy (per core, 4 images), vs the v1 PE-transpose kernel:
  - Activations reach channel-major with ZERO PE-transpose work:
      x f32 -> SBUF (SP DMA) -> DVE cast -> compact bf16 DRAM scratch
      (Pool/SWDGE) -> xbar-transpose DMA [1568pos,128ci] -> [128ci,1568]
      compact SBUF -> DVE re-stride into zero-padded 58x58 bf16 planes
      (one tile per image) -> DVE cast to fp8 planes.
    The PE queue holds ONLY conv matmuls (v1 spent ~60us of PE on 224
    transposes); conv data deps are DVE-only so no slow cross-engine
    event-semaphore chains gate the PE.
  - Mixed-precision conv to fit rel_err < 2e-2 (measured 1.789e-2):
      4 taps as fp8e4 DoubleRow matmuls (contraction 256 = both ci chunks
      per instruction, 2 MACs/cell/cycle; lhsT [128,2,128], rhs [128,2,464])
      + 5 taps as bf16 (exact +-1 weights; only bf16 rounding of x).
    Simulated quantization error scales as 2.65% * sqrt(n_fp8_pieces/18).
  - Moving operands are CONTIGUOUS full padded rows (DoubleRow needs 3D
    [Ki,Ko=2,dim] APs): psum [128co, 8rows*58] = 464 <= 512; the 2 junk
    cols per row (row-wrap) are dropped by the ACT psum->SBUF evacuation.
  - Outputs leave as bf16 (halves DMA; host upcasts to f32; adds ~0.2% err
    in quadrature). Queue split: SP = staging ins + xbars; Pool = stage
    outs + conv outs + memsets; ACT = weights + psum evacuation; DVE =
    casts/re-strides/sign.

Built with bacc.Bacc + nc.compile(); CoreSim no_exec time ~166us vs ~209us
for the v1 kernel (v1 measured 260972ns on HW).
"""

import numpy as np

import concourse.bacc as bacc
import concourse.mybir as mybir
import concourse.tile as tile

# ---- problem constants (hardcoded; kernel must be self-contained) ----
B_FULL, H, W, CI, CO, K = 32, 56, 56, 256, 256, 3
N_CORES = 8
B = B_FULL // N_CORES          # 4 images per core
IMG = H * W                    # 3136 positions per image
P = 128
HP, WP = H + 2, W + 2          # 58x58 zero-padded plane per image
IMGP = HP * WP                 # 3364
POSP = B * IMGP                # padded positions per core
CI_C = CI // P                 # 2 contraction chunks
CO_C = CO // P                 # 2 output-channel chunks
YCHUNK = 8                     # output rows per psum tile
NCHUNK = H // YCHUNK           # 7 chunks per image
# conv moving operands are CONTIGUOUS full padded rows (DoubleRow requires a
# 3D [Ki, Ko=2, dim] AP): psum covers 8 x 58 positions; 2 junk cols per row
# (row-wrap artifacts) are dropped at evacuation.
FREE = YCHUNK * WP             # 464 <= 512 psum fp32 bank limit
# per-image plane allocation, padded so (a) the last chunk's window stays
# in-bounds and (b) the cc stride is a 16 multiple (DoubleRow AP rule)
IMGPAD = IMGP + 28             # 3392, %16 == 0
QROWS = 14                     # image rows per staging piece (quarter image)
NQ = H // QROWS                # 4 pieces per image
STG_T = B * NQ                 # 16 staging pieces per core
QPOS = QROWS * W               # 784 positions per piece
PACK = 7                       # positions packed per staging partition row
# xbar halves: image rows [0,28) and [28,56) -> 1568 src rows each (%16==0)
NH = 2
HROWS = H // NH                # 28 image rows per half

# taps 0..FP8_NTAPS-1 run as fp8e4 DoubleRow; the rest bf16.
FP8_NTAPS = 4

F32 = mybir.dt.float32
BF16 = mybir.dt.bfloat16
FP8 = mybir.dt.float8e4


def _emit_body(nc, pools, x_d, xbf_d, w_d, o_d):
    (ws_pool, win_pool, stg_pool, stgo_pool, xc_pool, xt_pool, xt8_pool,
     out_pool, cpsum_pool) = pools

    x_flat = x_d.ap().flatten_outer_dims()        # [B*IMG, CI] f32
    xbf_flat = xbf_d.ap().flatten_outer_dims()    # [B*IMG, CI] bf16

    # ---- binarize weights: s bf16 + fp8, [128ci, 9tap*2cc, 256co] ----
    w_src = w_d.ap().rearrange("ky kx (cc p) co -> p (ky kx cc) co", p=P)
    wtile = win_pool.tile([P, K * K * CI_C, CO], F32, name="wtile")
    s_all = ws_pool.tile([P, K * K * CI_C, CO], BF16, name="s_all")
    s8_all = ws_pool.tile([P, K * K * CI_C, CO], FP8, name="s8_all")
    w_bounds = [0, 3, 6, 9, 12, 15, 18]

    def emit_weights():
        # emitted mid-staging (t==2) so the first stage casts lead the DVE
        # queue; w DMAs on ACT which is otherwise idle early
        for a, bnd in zip(w_bounds[:-1], w_bounds[1:]):
            nc.scalar.dma_start(out=wtile[:, a:bnd], in_=w_src[:, a:bnd])
        for a, bnd in zip(w_bounds[:-1], w_bounds[1:]):
            # sign(w) = 2*(w >= 0) - 1 (exact +-1 in bf16/fp8)
            nc.vector.tensor_scalar(
                s_all[:, a:bnd], wtile[:, a:bnd], 0.0, None, mybir.AluOpType.is_ge
            )
            nc.vector.tensor_scalar(
                s_all[:, a:bnd], s_all[:, a:bnd], 2.0, -1.0,
                mybir.AluOpType.mult, mybir.AluOpType.add,
            )
            nc.vector.tensor_copy(out=s8_all[:, a:bnd], in_=s_all[:, a:bnd])

    def s_tile(t, cc, oc):
        return s_all[:, t * CI_C + cc, oc * P : (oc + 1) * P]

    # (staging + xbar transposes are emitted interleaved below)

    # ---- channel-major activations: bf16 + fp8 padded 58x58 planes ----
    # One tile PER IMAGE: the dependency tracker bounds cc-spanning APs by
    # min-max byte range, so per-image tiles keep each matmul's deps to its
    # own image's casts instead of every image's.
    xts = [xt_pool.tile([P, CI_C, IMGPAD], BF16, name=f"xt{b}", tag=f"xt{b}")
           for b in range(B)]
    xt8s = [xt8_pool.tile([P, CI_C, IMGPAD], FP8, name=f"xt8{b}", tag=f"xt8{b}")
            for b in range(B)]
    xt_planes = [
        t[:, :, :IMGP].rearrange("p c (y x) -> p c y x", x=WP) for t in xts
    ]
    xt8_planes = [
        t[:, :, :IMGP].rearrange("p c (y x) -> p c y x", x=WP) for t in xt8s
    ]

    # zero the bf16 plane's pad strips (top/bottom rows, left/right cols);
    # the fp8 plane's pads come via the segment casts. Tail slack is read
    # (never used) by the last chunk's windows.
    for b in range(B):
        for cc in range(CI_C):
            nc.gpsimd.memset(xt_planes[b][:, cc, 0, :], 0.0)
            nc.gpsimd.memset(xt_planes[b][:, cc, HP - 1, :], 0.0)
            nc.gpsimd.memset(xt_planes[b][:, cc, 1 : HP - 1, 0], 0.0)
            nc.gpsimd.memset(xt_planes[b][:, cc, 1 : HP - 1, WP - 1], 0.0)
        nc.gpsimd.memset(xts[b][:, :, IMGP:], 0.0)
        nc.gpsimd.memset(xt8s[b][:, :, IMGP:], 0.0)

    # ---- staging + xbar transposes ----
    # Stage piece t=(b,q): x f32 [112, 7*256] -> SBUF -> DVE cast -> SWDGE
    # DMA (Pool) to compact DRAM xbf[b*IMG .. , ci] (contiguous, cheap).
    # xbar half h of (b, cc): xbf [1568pos, 128ci] -> COMPACT SBUF tile
    # [128, 1568] (2D both sides); DVE re-strides it into the padded bf16
    # plane (56 -> 58-wide rows) and casts the full padded segment to fp8.
    # Conv data deps are then DVE-only.
    stg_tiles = {}

    def emit_stage_in(t):
        b, q = divmod(t, NQ)
        st = stg_pool.tile([QPOS // PACK, PACK * CI], F32, name="stg", tag="stg")
        p0 = b * IMG + q * QPOS
        # image 0's odd pieces ingest via the (idle-at-start) ACT queue so
        # the first image lands in ~half the serial time; later pieces stay
        # on SP to keep ACT prompt for psum evacuation
        eng = nc.scalar if t in (1, 3) else nc.sync
        eng.dma_start(
            out=st, in_=x_flat[p0 : p0 + QPOS, :].rearrange(
                "(a b) c -> a (b c)", b=PACK)
        )
        stg_tiles[t] = st

    def emit_stage_out(t):
        b, q = divmod(t, NQ)
        so = stgo_pool.tile([QPOS // PACK, PACK * CI], BF16,
                            name="stgo", tag="stgo")
        nc.vector.tensor_copy(out=so, in_=stg_tiles.pop(t))
        p0 = b * IMG + q * QPOS
        nc.gpsimd.dma_start(
            out=xbf_flat[p0 : p0 + QPOS, :].rearrange(
                "(a b) c -> a (b c)", b=PACK),
            in_=so,
        )

    def emit_xbar(b, h):
        # both cc chunks: xbf [1568, 128] -> compact xc [128, 1568], then
        # DVE re-stride into plane rows [1+28h, 29+28h) cols 1..57
        p0 = b * IMG + h * HROWS * W
        for cc in range(CI_C):
            xc = xc_pool.tile([P, HROWS * W], BF16, name="xc", tag="xc")
            nc.sync.dma_start(
                out=xc,
                in_=xbf_flat[p0 : p0 + HROWS * W, cc * P : (cc + 1) * P],
                transpose=True,
            )
            nc.vector.tensor_copy(
                out=xt_planes[b][:, cc, 1 + h * HROWS : 1 + (h + 1) * HROWS,
                                 1 : 1 + W],
                in_=xc.rearrange("p (r c) -> p r c", c=W),
            )

    def emit_cast8(b, h):
        # cast the full padded-row segment (pads were memset in xt): half 0
        # covers padded rows [0, 29), half 1 [29, 58)
        s0 = 0 if h == 0 else 29 * WP
        s1 = 29 * WP if h == 0 else IMGP
        nc.vector.tensor_copy(
            out=xt8s[b][:, :, s0:s1],
            in_=xts[b][:, :, s0:s1],
        )

    # piece (b,q) covers image rows [14q, 14q+14): half (b,0) needs pieces
    # q<=1, half (b,1) needs q<=3. Emit one stage-out later so queue waits
    # are satisfied when reached.
    halves = [(b, h) for b in range(B) for h in (0, 1)]

    def half_need(bh):
        b, h = bh
        return b * NQ + (1 if h == 0 else 3)

    hi = 0
    for t in range(STG_T):
        emit_stage_in(t)
        if t == 4:
            emit_weights()
        if t >= 1:
            emit_stage_out(t - 1)
            while hi < len(halves) and half_need(halves[hi]) <= t - 1:
                emit_xbar(*halves[hi])
                emit_cast8(*halves[hi])
                hi += 1
    emit_stage_out(STG_T - 1)
    while hi < len(halves):
        emit_xbar(*halves[hi])
        emit_cast8(*halves[hi])
        hi += 1

    # ---- conv: per (image, row-chunk, oc) accumulate 3x3 taps in psum ----
    # Moving operand for tap (ky,kx) of chunk y0 is the CONTIGUOUS padded-
    # plane range starting at (y0+ky)*58 + kx, 464 long: psum element
    # (a, d) (d in [0,56)) = conv output at image row y0+a, col d; cols
    # 56,57 are row-wrap junk, dropped at evacuation.
    for b in range(B):
        for c in range(NCHUNK):
            y0 = c * YCHUNK
            for oc in range(CO_C):
                cps = cpsum_pool.tile([P, FREE], F32, name="cps", tag="cps")
                cps_rows = cps.rearrange("p (a d) -> p a d", d=WP)
                n_mm = FP8_NTAPS + (K * K - FP8_NTAPS) * CI_C
                mi = 0
                for t in range(FP8_NTAPS):
                    ky, kx = divmod(t, K)
                    start = (y0 + ky) * WP + kx
                    nc.tensor.matmul(
                        cps,
                        s8_all[:, t * CI_C : (t + 1) * CI_C,
                               oc * P : (oc + 1) * P],
                        xt8s[b][:, :, start : start + FREE],
                        start=(mi == 0),
                        stop=(mi == n_mm - 1),
                        perf_mode=mybir.MatmulPerfMode.DoubleRow,
                    )
                    mi += 1
                for t in range(FP8_NTAPS, K * K):
                    ky, kx = divmod(t, K)
                    start = (y0 + ky) * WP + kx
                    for cc in range(CI_C):
                        nc.tensor.matmul(
                            cps,
                            s_tile(t, cc, oc),
                            xts[b][:, cc, start : start + FREE],
                            start=(mi == 0),
                            stop=(mi == n_mm - 1),
                        )
                        mi += 1
                ot = out_pool.tile([P, YCHUNK * W], BF16, name="ot", tag="ot")
                nc.scalar.activation(
                    ot.rearrange("p (a d) -> p a d", d=W),
                    cps_rows[:, :, :W],
                    mybir.ActivationFunctionType.Copy,
                )
                # out DMA on Pool/SWDGE: ACT's queue stays short (psum copies
                # + event semaphores only) so the event sems PE waits on fire
                # promptly.
                nc.gpsimd.dma_start(
                    out=o_d.ap()[oc, :, b, y0 * W : (y0 + YCHUNK) * W],
                    in_=ot,
                )


def build_program(reps: int = 1):
    nc = bacc.Bacc("TRN2", debug=False, num_devices=N_CORES)
    x_d = nc.dram_tensor("x", [B, H, W, CI], F32, kind="ExternalInput")
    w_d = nc.dram_tensor("w", [K, K, CI, CO], F32, kind="ExternalInput")
    o_d = nc.dram_tensor("out", [CO_C, P, B, IMG], BF16, kind="ExternalOutput")
    xbf_d = nc.dram_tensor("xbf", [B, IMG, CI], BF16)

    with tile.TileContext(nc) as tc:
        with (
            tc.tile_pool(name="ws", bufs=1) as ws_pool,
            tc.tile_pool(name="win", bufs=1) as win_pool,
            tc.tile_pool(name="stg", bufs=4) as stg_pool,
            tc.tile_pool(name="stgo", bufs=4) as stgo_pool,
            tc.tile_pool(name="xcp", bufs=3) as xc_pool,
            tc.tile_pool(name="xtp", bufs=1) as xt_pool,
            tc.tile_pool(name="xt8p", bufs=1) as xt8_pool,
            tc.tile_pool(name="outs", bufs=10) as out_pool,
            tc.tile_pool(name="cpsum", bufs=8, space="PSUM") as cpsum_pool,
        ):
            pools = (ws_pool, win_pool, stg_pool, stgo_pool, xc_pool,
                     xt_pool, xt8_pool, out_pool, cpsum_pool)
            if reps == 1:
                _emit_body(nc, pools, x_d, xbf_d, w_d, o_d)
            else:
                with tc.For_i(0, reps, 1):
                    _emit_body(nc, pools, x_d, xbf_d, w_d, o_d)
    nc.compile()
    return nc


_NC_CACHE = {}


def _get_program(reps: int = 1):
    if reps not in _NC_CACHE:
        _NC_CACHE[reps] = build_program(reps)
    return _NC_CACHE[reps]


def kernel(x: np.ndarray, w: np.ndarray) -> np.ndarray:
    from concourse.bass_utils import run_bass_kernel_spmd

    x = np.ascontiguousarray(x, dtype=np.float32)
    w = np.ascontiguousarray(w, dtype=np.float32)
    nc = _get_program()
    in_maps = [
        {"x": np.ascontiguousarray(x[c * B : (c + 1) * B]), "w": w}
        for c in range(N_CORES)
    ]
    res = run_bass_kernel_spmd(nc, in_maps, core_ids=list(range(N_CORES))).results
    outs = []
    for c in range(N_CORES):
        r = np.asarray(res[c]["out"]).astype(np.float32)  # (CO_C, P, B, IMG)
        o = r.transpose(2, 3, 0, 1).reshape(B, H, W, CO)
        outs.append(o)
    return np.ascontiguousarray(np.concatenate(outs, axis=0))
